# revision 31
# baseline (speedup 1.0000x reference)
"""Trainium2 Bass kernel for a 2-layer GCN (Cora-style GNN message passing).

Computation (see reference):
    S1 = x @ W1                      # [N, 40]
    agg1[d] = sum_e w_e * S1[src_e]  (segment-sum over dst) + b1
    h = relu(agg1) * keep            # keep = (dropout_mask > 0.5) / 0.5
    S2 = h @ W2                      # [N, 7]
    agg2[d] = sum_e w_e * S2[src_e]  + b2
    out = log_softmax(agg2, axis=1)

Distribution (8 NeuronCores): nodes sharded by dst range; each core owns
12,500 nodes and all edges whose dst falls in its range. Each core computes
S1/S2 rows for its own nodes, the tables are all-gathered (fp16), and the
per-core segment-sum is an indirect-DMA gather of src rows plus one-hot
matmuls on the tensor engine:

  - edges are sorted by dst and packed into groups of 128 (partition dim),
    each group confined to a 32-dst window,
  - the weighted one-hot [128 edges, 32 slots] fp16 is built ON DEVICE from
    per-edge (slot, weight) via is_equal against an iota + multiply, so only
    3 fp16/i32 values ship per edge instead of a 32-wide one-hot row,
  - windows accumulate into a [32 slots, 4 windows, width] PSUM tile.

x ships as int8 (symmetric quantization, scale folded into W1); the device
converts int8 -> fp16 ahead of the tensor engine. Tables are fp16 with no
padding rows, so the gather index of a src node is just the global node id.

All group counts are unified across cores so the single SPMD program works
on every core; padding edges carry weight 0.
"""

import os
import numpy as np
from dataclasses import dataclass

try:  # persistent XLA compilation cache: the warm-up run pre-compiles
    import jax as _jax
    _jax.config.update("jax_compilation_cache_dir", "/tmp/jaxcache")
    _jax.config.update("jax_persistent_cache_min_compile_time_secs", 0.0)
    _jax.config.update("jax_persistent_cache_min_entry_size_bytes", 0)
except Exception:
    pass

f16 = np.float16

XQ_SCALE = 4.35 / 127.0  # int8 quantization step for x (clip at +-4.35 sigma)


@dataclass(frozen=True)
class Cfg:
    ncores: int = 8
    own: int = 12500          # real nodes per core (no padding in tables)
    nout: int = 12544         # output rows per core (98 tiles of 128)
    feat: int = 1433
    fpad: int = 1440          # feat padded to 12 chunks of 120
    kp: int = 120             # feat-chunk partition size
    hid: int = 40
    ncls: int = 7
    win: int = 32             # dst nodes per window (one-hot width)
    wpt: int = 4              # windows per 128-node tile (128/win)

    @property
    def tiles(self):
        return self.nout // 128

    @property
    def windows(self):
        return self.nout // self.win  # per core

    @property
    def kt(self):
        return self.fpad // self.kp

    @property
    def n(self):
        return self.ncores * self.own


CFG = Cfg()


# --------------------------------------------------------------------------
# Host-side preprocessing
# --------------------------------------------------------------------------

def host_prep(cfg, x, src, dst, edge_weight, W1, b1, W2, b2, dropout_mask_u):
    """Build per-core input arrays + the (core-invariant) group structure."""
    ncores, own, win = cfg.ncores, cfg.own, cfg.win
    nwin = cfg.windows

    src = np.asarray(src).astype(np.int64)
    dst = np.asarray(dst).astype(np.int64)
    core = dst // own
    ldst = dst - core * own
    wloc = ldst // win                      # window within core [0, nwin)
    slot = ldst - wloc * win                # [0, win)
    gwin = core * nwin + wloc               # global window id

    nwin_total = ncores * nwin
    cnt = np.bincount(gwin, minlength=nwin_total).reshape(ncores, nwin)
    # unified groups-per-window across cores (>=1 so every psum slab is written)
    Gw = np.maximum(1, -(-cnt // 128)).max(axis=0)          # [nwin]
    woff = np.concatenate([[0], np.cumsum(Gw)])             # group offsets
    G = int(woff[-1])

    # stable sort edges by global window; position within window
    order = np.argsort(gwin, kind="stable")
    gw_sorted = gwin[order]
    grp_start = np.concatenate(
        [[0], np.cumsum(np.bincount(gwin, minlength=nwin_total))]
    )
    pos_in_win = np.arange(len(src)) - grp_start[gw_sorted]
    # per-core padded edge position
    tgt = woff[gw_sorted % nwin] * 128 + pos_in_win

    idx_cores = np.zeros((ncores, G * 128), np.int32)
    w_cores = np.zeros((ncores, G * 128), f16)
    ew = np.asarray(edge_weight).astype(f16)
    c_sorted = gw_sorted // nwin
    packed = (src | (slot << 20)).astype(np.int32)  # src in low 20 bits
    for k in range(ncores):
        m = c_sorted == k
        idx_cores[k, tgt[m]] = packed[order[m]]
        w_cores[k, tgt[m]] = ew[order[m]]

    # SBUF layouts [128, G]: partition = position within group
    def to_pg(a):
        return np.ascontiguousarray(
            a.reshape(ncores, G, 128).transpose(0, 2, 1)
        )

    gidx = to_pg(idx_cores)
    wf = to_pg(w_cores)

    # per-core xq [fpad, own] int8 (x / XQ_SCALE, clipped)
    xs = np.asarray(x, dtype=np.float32) * (1.0 / XQ_SCALE)
    np.clip(np.rint(xs), -127, 127, out=xs)
    xs = xs.astype(np.int8)
    xq = np.zeros((ncores, cfg.fpad, own), np.int8)
    for k in range(ncores):
        xq[k, : cfg.feat, :] = xs[k * own:(k + 1) * own].T

    # W1 packed [kp, kt, hid] fp16, pre-scaled by the int8 step
    w1p = np.zeros((cfg.fpad, cfg.hid), np.float32)
    w1p[: cfg.feat] = np.asarray(W1, np.float32) * XQ_SCALE
    w1p = np.ascontiguousarray(
        w1p.reshape(cfg.kt, cfg.kp, cfg.hid).transpose(1, 0, 2)
    ).astype(f16)

    # keep mask {0,1}, transposed: [hid, nout] fp16; the /(1-p)=x2 dropout
    # scale is folded into W2
    keepT = np.zeros((ncores, cfg.hid, cfg.nout), f16)
    keep01 = (np.asarray(dropout_mask_u) > 0.5).astype(f16)
    for k in range(ncores):
        keepT[k, :, :own] = keep01[k * own:(k + 1) * own].T

    b1c = np.asarray(b1, np.float32).reshape(cfg.hid, 1).copy()
    b2f = np.broadcast_to(
        np.asarray(b2, np.float32), (cfg.win, cfg.wpt, cfg.ncls)
    ).copy()
    w2 = np.asarray(W2, np.float32) * 2.0

    in_maps = [
        {
            "xq": xq[k],
            "w1p": w1p,
            "w2": w2,
            "b1c": b1c,
            "b2f": b2f,
            "keepT": keepT[k],
            "gidx": gidx[k],
            "wf": wf[k],
        }
        for k in range(ncores)
    ]
    return in_maps, Gw


# --------------------------------------------------------------------------
# Numpy emulation of the device algorithm (for validation)
# --------------------------------------------------------------------------

def emulate(cfg, in_maps, Gw):
    f32 = np.float32
    ncores, own, win, hid, ncls = cfg.ncores, cfg.own, cfg.win, cfg.hid, cfg.ncls
    G = int(Gw.sum())
    woff = np.concatenate([[0], np.cumsum(Gw)])

    # phase A: S1 tables
    s1 = np.zeros((ncores, own, hid), f16)
    for k in range(ncores):
        xb = in_maps[k]["xq"].astype(f32)           # [fpad, own] (int values)
        w1p = in_maps[k]["w1p"].astype(f32)         # [kp, kt, hid]
        acc = np.zeros((own, hid), f32)
        for kk in range(cfg.kt):
            acc += xb[kk * cfg.kp:(kk + 1) * cfg.kp].T @ w1p[:, kk, :]
        s1[k] = acc.astype(f16)
    s1_full = s1.reshape(ncores * own, hid)

    def spmm(table, width):
        aggs = np.zeros((ncores, cfg.nout, width), f32)
        for k in range(ncores):
            pk = in_maps[k]["gidx"]                 # [128, G] packed
            gidx = pk & 0xFFFFF
            sla = (pk >> 20).astype(f32)
            wfa = in_maps[k]["wf"].astype(f32)      # [128, G]
            msg = table[gidx.T.ravel()].astype(f32).reshape(G, 128, width)
            oh = (sla[:, :, None] == np.arange(win)[None, None, :])
            oh = (oh * wfa[:, :, None]).astype(f16).astype(f32)  # [128,G,win]
            for w in range(cfg.windows):
                t, wl = divmod(w, cfg.wpt)
                base = t * 128 + wl * win
                for g in range(woff[w], woff[w + 1]):
                    aggs[k, base:base + win] += oh[:, g, :].T @ msg[g]
        return aggs

    agg1 = spmm(s1_full, hid)
    s2 = np.zeros((ncores, own, ncls), f16)
    for k in range(ncores):
        b1 = in_maps[k]["b1c"][:, 0]
        h = np.maximum(agg1[k] + b1, 0.0) * in_maps[k]["keepT"].T.astype(f32)
        s2[k] = (h[:own] @ in_maps[k]["w2"]).astype(f16)
    s2_full = s2.reshape(ncores * own, ncls)

    agg2 = spmm(s2_full, ncls)
    outs = []
    for k in range(ncores):
        z = agg2[k] + in_maps[k]["b2f"][0, 0]
        m = z.max(1, keepdims=True)
        out = (z - m) - np.log(np.exp(z - m).sum(1, keepdims=True))
        outs.append(out[:own].astype(f16))          # device ships fp16 output
    return np.concatenate(outs).astype(np.float32)


# --------------------------------------------------------------------------
# Bass/Tile program
# --------------------------------------------------------------------------

def build_program(cfg, Gw, num_devices):
    import concourse.bass as bass
    import concourse.bacc as bacc
    import concourse.mybir as mybir
    import concourse.tile as tile
    from concourse.masks import make_identity

    f32 = mybir.dt.float32
    fp16 = mybir.dt.float16
    i8 = mybir.dt.int8
    i32 = mybir.dt.int32
    AF = mybir.ActivationFunctionType
    OP = mybir.AluOpType
    X = mybir.AxisListType.X

    G = int(Gw.sum())
    woff = np.concatenate([[0], np.cumsum(Gw)])
    own, nout, tiles = cfg.own, cfg.nout, cfg.tiles
    win, wpt, hid, ncls, kt, kp = cfg.win, cfg.wpt, cfg.hid, cfg.ncls, cfg.kt, cfg.kp
    trows = num_devices * own

    nc = bacc.Bacc(
        "TRN2", target_bir_lowering=False, debug=False,
        num_devices=num_devices,
    )

    xq = nc.dram_tensor("xq", [cfg.fpad, own], i8, kind="ExternalInput")
    w1p = nc.dram_tensor("w1p", [kp, kt, hid], fp16, kind="ExternalInput")
    w2 = nc.dram_tensor("w2", [hid, ncls], f32, kind="ExternalInput")
    b1c = nc.dram_tensor("b1c", [hid, 1], f32, kind="ExternalInput")
    b2f = nc.dram_tensor("b2f", [win, wpt, ncls], f32, kind="ExternalInput")
    keepT = nc.dram_tensor("keepT", [hid, nout], fp16, kind="ExternalInput")
    gidx_d = nc.dram_tensor("gidx", [128, G], i32, kind="ExternalInput")
    wf_d = nc.dram_tensor("wf", [128, G], fp16, kind="ExternalInput")
    rmax = int(max(woff[(t + 1) * wpt] - woff[t * wpt] for t in range(tiles)))
    out_d = nc.dram_tensor("out", [nout, ncls], fp16, kind="ExternalOutput")

    s1_own = nc.dram_tensor("s1_own", [own, hid], fp16)
    s1_full = nc.dram_tensor("s1_full", [trows, hid], fp16, addr_space="Shared")
    s2_own = nc.dram_tensor("s2_own", [own, ncls], fp16)
    s2_full = nc.dram_tensor("s2_full", [trows, ncls], fp16, addr_space="Shared")

    groups = list(range(num_devices))

    # per-tile group schedule: (g_global, window_in_tile, start, stop)
    sched = []
    for t in range(tiles):
        entries = []
        for wl in range(wpt):
            w = t * wpt + wl
            for j, g in enumerate(range(woff[w], woff[w + 1])):
                entries.append(
                    (int(g), wl, j == 0, g == woff[w + 1] - 1)
                )
        sched.append(entries)

    # node chunks for phase A (own = 97*128 + 84)
    nchunks = [(c * 128, min(128, own - c * 128)) for c in range((own + 127) // 128)]

    with tile.TileContext(nc) as tc:
        with (
            tc.tile_pool(name="const", bufs=1) as constp,
            tc.tile_pool(name="xbuf", bufs=3) as xpool,
            tc.tile_pool(name="psA", bufs=2, space="PSUM") as psA,
            tc.tile_pool(name="s1pc", bufs=3) as s1pool,
            tc.tile_pool(name="msg", bufs=3) as msgp,
            tc.tile_pool(name="ohb", bufs=3) as ohp,
            tc.tile_pool(name="psB", bufs=2, space="PSUM") as psB,
            tc.tile_pool(name="hb", bufs=3) as hpool,
            tc.tile_pool(name="psT", bufs=2, space="PSUM") as psT,
            tc.tile_pool(name="ps2", bufs=2, space="PSUM") as ps2,
            tc.tile_pool(name="ob", bufs=3) as opool,
        ):
            # ---- constants ----
            w1sb = constp.tile([kp, kt, hid], fp16)
            nc.sync.dma_start(out=w1sb[:], in_=w1p[:])
            w2sb = constp.tile([hid, ncls], f32)
            nc.sync.dma_start(out=w2sb[:], in_=w2[:])
            b1sb = constp.tile([hid, 1], f32)
            nc.sync.dma_start(out=b1sb[:], in_=b1c[:])
            b2sb = constp.tile([win, wpt, ncls], f32)
            nc.sync.dma_start(out=b2sb[:], in_=b2f[:])
            ident = constp.tile([128, 128], f32)
            make_identity(nc, ident[:])
            # edge metadata, resident for both layers: the shipped gidx packs
            # slot<<20 | src; unpack on device
            pk = constp.tile([128, G], i32)
            nc.sync.dma_start(out=pk[:], in_=gidx_d[:])
            wsb = constp.tile([128, G], fp16)
            nc.sync.dma_start(out=wsb[:], in_=wf_d[:])
            gidx = constp.tile([128, G], i32)
            nc.vector.tensor_scalar(
                out=gidx[:], in0=pk[:], scalar1=0xFFFFF, scalar2=None,
                op0=OP.bitwise_and,
            )
            sh = constp.tile([128, G], i32)
            nc.vector.tensor_scalar(
                out=sh[:], in0=pk[:], scalar1=20, scalar2=None,
                op0=OP.logical_shift_right,
            )
            slsb = constp.tile([128, G], fp16)
            nc.vector.tensor_copy(slsb[:], sh[:])
            # iota over the one-hot slots, materialized on device
            ioi = constp.tile([128, rmax, win], i32)
            nc.gpsimd.iota(
                ioi[:], pattern=[[0, rmax], [1, win]], base=0,
                channel_multiplier=0,
            )
            iosb = constp.tile([128, rmax, win], fp16)
            nc.vector.tensor_copy(iosb[:], ioi[:])
            # keep mask, converted once to f32 for the f32 multiply
            kp16 = constp.tile([hid, nout], fp16)
            nc.sync.dma_start(out=kp16[:], in_=keepT[:])
            kp32 = constp.tile([hid, nout], f32)
            nc.vector.tensor_copy(kp32[:], kp16[:])

            # ---- phase A: S1_own = (x @ W1) per 128-node chunk ----
            xq_r = xq[:].rearrange("(k p) n -> p k n", p=kp)
            for c0, cw in nchunks:
                xt8 = xpool.tile([kp, kt, 128], i8, tag="x8")
                nc.sync.dma_start(
                    out=xt8[:, :, :cw], in_=xq_r[:, :, c0:c0 + cw]
                )
                xt = xpool.tile([kp, kt, 128], fp16, tag="x16")
                nc.vector.tensor_copy(xt[:, :, :cw], xt8[:, :, :cw])
                ps = psA.tile([128, hid], f32)
                for k in range(kt):
                    nc.tensor.matmul(
                        ps[:cw, :], lhsT=xt[:, k, :cw], rhs=w1sb[:, k, :],
                        start=(k == 0), stop=(k == kt - 1),
                    )
                pc = s1pool.tile([128, hid], fp16, tag="s1pc")
                nc.vector.tensor_copy(pc[:cw, :], ps[:cw, :])
                nc.sync.dma_start(
                    out=s1_own[c0:c0 + cw, :], in_=pc[:cw, :]
                )

            # ---- all-gather S1 ----
            nc.gpsimd.collective_compute(
                "AllGather", OP.bypass, replica_groups=[groups],
                ins=[s1_own[:]], outs=[s1_full[:]],
            )

            def build_onehot(t, r0, rt):
                """[128, rt, win] fp16 weighted one-hot for tile t's groups."""
                oht = ohp.tile([128, rmax, win], fp16, tag="oh")
                nc.vector.tensor_tensor(
                    out=oht[:, :rt, :],
                    in0=slsb[:, r0:r0 + rt].to_broadcast([128, rt, win]),
                    in1=iosb[:, :rt, :],
                    op=OP.is_equal,
                )
                nc.vector.tensor_tensor(
                    out=oht[:, :rt, :],
                    in0=oht[:, :rt, :],
                    in1=wsb[:, r0:r0 + rt].to_broadcast([128, rt, win]),
                    op=OP.mult,
                )
                return oht

            # ---- layer 1 SpMM -> h^T -> S2_own ----
            for t in range(tiles):
                r0 = int(woff[t * wpt])
                rt = int(woff[(t + 1) * wpt]) - r0
                c0 = t * 128
                cw = min(128, own - c0)
                msg = msgp.tile([128, rmax, hid], fp16, tag="msg1")
                # funnel the gather's WAR dependency on msg through a cheap
                # Pool-engine op first
                nc.gpsimd.memset(msg[:1, :1, :1], 0.0)
                # HW only supports one offset per partition per indirect DMA
                for r in range(rt):
                    nc.gpsimd.indirect_dma_start(
                        out=msg[:, r, :], out_offset=None,
                        in_=s1_full[:],
                        in_offset=bass.IndirectOffsetOnAxis(
                            ap=gidx[:, r0 + r:r0 + r + 1], axis=0
                        ),
                    )
                oht = build_onehot(t, r0, rt)
                ps = psB.tile([win, wpt, hid], f32, tag="agg")
                for (g, wl, st, sp) in sched[t]:
                    r = g - r0
                    nc.tensor.matmul(
                        ps[:, wl, :],
                        lhsT=oht[:, r, :], rhs=msg[:, r, :],
                        start=st, stop=sp,
                    )
                agg_sb = hpool.tile([win, wpt, hid], f32, tag="agg_sb")
                nc.vector.tensor_copy(agg_sb[:], ps[:])
                pst = psT.tile([hid, wpt, win], f32, tag="hT")
                for wl in range(wpt):
                    nc.tensor.transpose(
                        pst[:, wl, :], agg_sb[:, wl, :], ident[:win, :win]
                    )
                hT = hpool.tile([hid, 128], f32, tag="hT_sb")
                nc.scalar.activation(
                    out=hT[:],
                    in_=pst[:].rearrange("p w s -> p (w s)"),
                    func=AF.Relu, bias=b1sb[:], scale=1.0,
                )
                nc.vector.tensor_tensor(
                    out=hT[:], in0=hT[:],
                    in1=kp32[:, t * 128:(t + 1) * 128], op=OP.mult
                )
                p2 = ps2.tile([128, ncls], f32, tag="s2")
                nc.tensor.matmul(
                    p2[:], lhsT=hT[:], rhs=w2sb[:], start=True, stop=True
                )
                s2pc = s1pool.tile([128, ncls], fp16, tag="s2pc")
                nc.vector.tensor_copy(s2pc[:cw, :], p2[:cw, :])
                nc.sync.dma_start(
                    out=s2_own[c0:c0 + cw, :], in_=s2pc[:cw, :]
                )

            # ---- all-gather S2 ----
            nc.gpsimd.collective_compute(
                "AllGather", OP.bypass, replica_groups=[groups],
                ins=[s2_own[:]], outs=[s2_full[:]],
            )

            # ---- layer 2 SpMM + log_softmax ----
            out_r = out_d[:].rearrange("(t w s) c -> t s w c", s=win, w=wpt)
            for t in range(tiles):
                r0 = int(woff[t * wpt])
                rt = int(woff[(t + 1) * wpt]) - r0
                msg = msgp.tile([128, rmax, ncls], fp16, tag="msg2")
                nc.gpsimd.memset(msg[:1, :1, :1], 0.0)
                for r in range(rt):
                    nc.gpsimd.indirect_dma_start(
                        out=msg[:, r, :], out_offset=None,
                        in_=s2_full[:],
                        in_offset=bass.IndirectOffsetOnAxis(
                            ap=gidx[:, r0 + r:r0 + r + 1], axis=0
                        ),
                    )
                oht = build_onehot(t, r0, rt)
                ps = psB.tile([win, wpt, ncls], f32, tag="agg")
                for (g, wl, st, sp) in sched[t]:
                    r = g - r0
                    nc.tensor.matmul(
                        ps[:, wl, :],
                        lhsT=oht[:, r, :], rhs=msg[:, r, :],
                        start=st, stop=sp,
                    )
                z = opool.tile([win, wpt, ncls], f32, tag="z")
                nc.vector.tensor_tensor(
                    out=z[:], in0=ps[:], in1=b2sb[:], op=OP.add
                )
                m = opool.tile([win, wpt], f32, tag="m")
                nc.vector.tensor_reduce(out=m[:], in_=z[:], axis=X, op=OP.max)
                zc = opool.tile([win, wpt, ncls], f32, tag="zc")
                nc.vector.tensor_tensor(
                    out=zc[:], in0=z[:],
                    in1=m[:].to_broadcast([win, wpt, ncls]), op=OP.subtract,
                )
                ez = opool.tile([win, wpt, ncls], f32, tag="ez")
                nc.scalar.activation(out=ez[:], in_=zc[:], func=AF.Exp)
                s = opool.tile([win, wpt], f32, tag="s")
                nc.vector.tensor_reduce(out=s[:], in_=ez[:], axis=X, op=OP.add)
                ls = opool.tile([win, wpt], f32, tag="ls")
                nc.scalar.activation(out=ls[:], in_=s[:], func=AF.Ln)
                res = opool.tile([win, wpt, ncls], fp16, tag="res")
                nc.vector.tensor_tensor(
                    out=res[:], in0=zc[:],
                    in1=ls[:].to_broadcast([win, wpt, ncls]), op=OP.subtract,
                )
                nc.sync.dma_start(out=out_r[t], in_=res[:])

    nc.compile()
    return nc


# --------------------------------------------------------------------------
# Entry point
# --------------------------------------------------------------------------

def kernel(x, src, dst, edge_weight, W1, b1, W2, b2, dropout_mask_u):
    cfg = CFG
    in_maps, Gw = host_prep(
        cfg, x, src, dst, edge_weight, W1, b1, W2, b2, dropout_mask_u
    )
    nc = build_program(cfg, Gw, cfg.ncores)

    from concourse.bass_utils import run_bass_kernel_spmd

    trace = bool(int(os.environ.get("GNN_TRACE", "0")))
    try:
        res = run_bass_kernel_spmd(
            nc, in_maps, core_ids=list(range(cfg.ncores)), trace=trace
        )
    except ModuleNotFoundError:
        res = run_bass_kernel_spmd(
            nc, in_maps, core_ids=list(range(cfg.ncores)), trace=False
        )
    kernel.last_exec_time_ns = getattr(res, "exec_time_ns", None)
    kernel.last_profile = res
    out = np.concatenate(
        [res.results[k]["out"][: cfg.own] for k in range(cfg.ncores)]
    )
    return out.astype(np.float32)
